# revision 7
# baseline (speedup 1.0000x reference)
"""Trainium2 Bass kernel for nn_Attention_3513283248742.

Bilinear attention: scores = h @ W @ b^T, attn = softmax(scores, -1),
ctx = attn @ b.  Shapes: b [32,1024,1024], h [32,256,1024], W_b [1,1024,1024].

Sharding: data-parallel over batch B=32 across 8 NeuronCores (4 batches per
core); W replicated.  No collectives.

v2 over the previous kernel: all hT / bT transposes move to the HOST (the
inputs are sent both in natural and transposed layout, fp16), removing 80
PE transposes per batch (~17us/core of PE time).  Output DMA'd as fp16 and
upcast on the host.  Per-core DMA grows to ~22MB (~62us) which stays under
the fp16 matmul floor (~85us), so the kernel remains PE-bound.

Per-core pipeline (per batch i):
  hWT  = W^T @ hT_i           lhsT = W chunks, rhs = hT (from host)
  S    = hWT^T @ bT_i         scores [q,k]
  softmax over k: DVE row max, ACT exp (+rowsum via accum), DVE recip
  attnT = PE transposes of E (the only transposes left on the PE)
  ctx  = attnT^T @ b_i        rhs = b natural layout (from host)
  out  = ctx * invS           ACT epilogue, fp16, DMA'd from the ACT queue

hWT(i+1) matmul groups are interleaved into batch i's attnT/ctx stream as
PE filler so softmax / PSUM->SBUF copy latency never stalls the PE.  Batch
0's hWT runs j-outer (contraction-chunk outer) so its matmuls chase the W
chunk DMAs during the ramp.
"""

import numpy as np

import concourse.bass as bass
import concourse.mybir as mybir
import concourse.tile as tile
from concourse.bass_utils import run_bass_kernel_spmd
from concourse.vector_clock import ScopedClock

F32 = mybir.dt.float32
F16 = mybir.dt.float16

N_CORES = 8
B, TB, TH, D = 32, 1024, 256, 1024
BPC = B // N_CORES  # batches per core = 4
P = 128
NDC = D // P   # 8 chunks of the D axis
NKC = TB // P  # 8 chunks of the k axis
NQ = TH // P   # 2 chunks of the q axis

_PATCHED = False
CLEAR_SEMS_ON_EXIT = False


def _patch_tile_drain(max_waits_per_inst: int = 1):
    """This walrus build rejects >1 sem wait on the SP Drain instruction that
    TileContext emits on exit; split the waits across preceding sync nops."""
    global _PATCHED
    if _PATCHED:
        return
    _PATCHED = True

    def _drain_and_barrier(self, tick_clock, wait_clock):
        nc = self.nc
        drain_inst = nc.sync.drain()
        wait_clock.add_sem_waits(
            drain_inst.ins, ScopedClock({None: tick_clock.global_clock})
        )
        si = drain_inst.ins.sync_info
        if si is not None and si.on_wait and len(si.on_wait) > max_waits_per_inst:
            waits = list(si.on_wait)
            bb = nc.cur_bb.bb
            assert bb.instructions[-1] is drain_inst.ins
            bb.instructions.pop()
            si.on_wait = waits[:max_waits_per_inst]
            rest = waits[max_waits_per_inst:]
            for i in range(0, len(rest), max_waits_per_inst):
                nop = nc.sync.nop(nofuse=True)
                chunk = rest[i : i + max_waits_per_inst]
                if nop.ins.sync_info is None:
                    nop.ins.sync_info = mybir.SyncInfo(on_wait=chunk, on_update=[])
                else:
                    nop.ins.sync_info.on_wait.extend(chunk)
            bb.instructions.append(drain_inst.ins)
        nc.all_engine_barrier()
        assert self.sems is not None
        popped = nc._tile_sem_poison_stack.pop()
        assert popped is self._sem_poison
        if CLEAR_SEMS_ON_EXIT:
            nc.clear_and_free_semaphores(list(self.sems.allocated().values()))
            nc.all_engine_barrier()
        else:
            nc._state.prepend_free_semaphores(
                [
                    s.num if hasattr(s, "num") else s
                    for s in self.sems.allocated().values()
                ]
            )

    tile.TileContext._drain_and_barrier = _drain_and_barrier


def _split_excess_waits(nc, max_waits: int = 1):
    """Walrus rejects instructions carrying more than `max_waits` sem waits.
    Hoist excess waits onto same-engine nops inserted just before."""
    for f in nc.m.functions:
        for bb in f.blocks:
            out = []
            for ins in list(bb.instructions):
                si = ins.sync_info
                if si is not None and si.on_wait and len(si.on_wait) > max_waits:
                    waits = list(si.on_wait)
                    si.on_wait = waits[:max_waits]
                    rest = waits[max_waits:]
                    for i in range(0, len(rest), max_waits):
                        nop = nc.engines[ins.engine].nop(nofuse=True)
                        cur_bb = nc.cur_bb.bb
                        assert cur_bb.instructions[-1] is nop.ins
                        cur_bb.instructions.pop()
                        nop.ins.sync_info = mybir.SyncInfo(
                            on_wait=rest[i : i + max_waits], on_update=[]
                        )
                        out.append(nop.ins)
                out.append(ins)
            bb.instructions[:] = out


def build_nc():
    _patch_tile_drain()
    nc = bass.Bass(trn_type="TRN2", target_bir_lowering=False, debug=False)
    b_ext = nc.declare_dram_parameter("b", [BPC, TB, D], F16, isOutput=False)
    bt_ext = nc.declare_dram_parameter("bT", [BPC, D, TB], F16, isOutput=False)
    ht_ext = nc.declare_dram_parameter("hT", [BPC, D, TH], F16, isOutput=False)
    w_ext = nc.declare_dram_parameter("w", [D, D], F16, isOutput=False)
    ident_ext = nc.declare_dram_parameter("ident", [P, P], F16, isOutput=False)
    out_ext = nc.declare_dram_parameter("out", [BPC, TH, D], F16, isOutput=True)

    with tile.TileContext(nc) as tc:
        with (
            tc.tile_pool(name="consts", bufs=1) as consts,
            tc.tile_pool(name="bpool", bufs=2) as bpool,
            tc.tile_pool(name="btpool", bufs=2) as btpool,
            tc.tile_pool(name="htpool", bufs=2) as htpool,
            tc.tile_pool(name="hwtpool", bufs=2) as hwtpool,
            tc.tile_pool(name="epool", bufs=2) as epool,
            tc.tile_pool(name="atpool", bufs=2) as atpool,
            tc.tile_pool(name="ctxpool", bufs=2) as ctxpool,
            tc.tile_pool(name="stats", bufs=2) as stats,
            tc.tile_pool(name="psbig", bufs=2, space="PSUM") as psbig,
            tc.tile_pool(name="pshw", bufs=2, space="PSUM") as pshw,
            tc.tile_pool(name="psT", bufs=2, space="PSUM") as psT,
        ):
            # ident on the (otherwise idle at t=0) scalar DMA queue so the
            # warmup transposes can start as soon as the preamble ends.
            ident_t = consts.tile([P, P], F16)
            nc.scalar.dma_start(ident_t[:], ident_ext.ap())
            ident16 = ident_t[:]

            w16 = consts.tile([P, NDC, D], F16)  # [din(part), j, dout]

            # --- DMA emission helpers (sync queue = priority load stream) ---
            def load_ht(i):
                t = htpool.tile([P, NDC, TH], F16, name=f"hT{i}", tag="hT")
                nc.sync.dma_start(t[:], ht_ext[i].rearrange("(c p) q -> p c q", p=P))
                return t

            def load_bt(i, halves):
                t = btpool.tile([P, NDC, TB], F16, name=f"bT{i}", tag="bT")
                if halves:
                    for kh in range(2):
                        nc.sync.dma_start(
                            t[:, :, kh * 512 : (kh + 1) * 512],
                            bt_ext[i, :, kh * 512 : (kh + 1) * 512].rearrange(
                                "(c p) k -> p c k", p=P
                            ),
                        )
                else:
                    nc.sync.dma_start(
                        t[:], bt_ext[i].rearrange("(c p) k -> p c k", p=P)
                    )
                return t

            def load_b(i):
                t = bpool.tile([P, NKC, D], F16, name=f"b{i}", tag="b")
                nc.sync.dma_start(t[:], b_ext[i].rearrange("(c p) d -> p c d", p=P))
                return t

            # --- ramp: priority-ordered loads on the sync queue ---
            hT = [None] * (BPC + 1)
            bT = [None] * BPC
            bN = [None] * BPC
            hT[0] = load_ht(0)
            for j2 in range(4):  # W in 4 chunks so hWT(0) can chase arrivals
                nc.sync.dma_start(
                    w16[:, 2 * j2 : 2 * j2 + 2, :],
                    w_ext[j2 * 256 : (j2 + 1) * 256, :].rearrange(
                        "(c p) d -> p c d", p=P
                    ),
                )
            bT[0] = load_bt(0, halves=True)
            hT[1] = load_ht(1)
            bN[0] = load_b(0)

            # --- PE warmup: ramp the clock while the ramp DMAs stream ---
            for wi in range(12):
                wt = psT.tile([P, TB], F16, name="warm", tag="ps16")
                nc.tensor.transpose(
                    wt[:, (wi % 8) * P : (wi % 8 + 1) * P], ident16, ident16
                )

            hWT = [None] * (BPC + 1)
            hWT[0] = hwtpool.tile([P, NDC, TH], F16, name="hWT0", tag="hWT")

            def emit_hwt_group(i, tp):
                """One tp-group (2 dout chunks) of hWT for batch i. 16 mm."""
                ps = pshw.tile([P, 512], F32, name="ps_hw", tag="pshw")
                for dt in range(2):
                    t = tp + dt
                    for j in range(NDC):
                        nc.tensor.matmul(
                            ps[:, dt * 256 : (dt + 1) * 256],
                            w16[:, j, t * P : (t + 1) * P],
                            hT[i][:, j, :],
                            start=(j == 0),
                            stop=(j == NDC - 1),
                        )
                nc.vector.tensor_copy(
                    hWT[i][:, tp : tp + 2, :].rearrange("p a b -> p (a b)"),
                    ps[:],
                )

            # hWT for batch 0 during the ramp: contiguous accumulation
            # groups; each matmul's W-chunk DMA dependency still lets the
            # stream chase the W arrivals.
            for tp in range(0, NDC, 2):
                emit_hwt_group(0, tp)

            # --- per-batch emission ---
            def make_batch(i):
                E = epool.tile([P, NQ, TB], F16, name=f"E{i}", tag="E")
                negmax = stats.tile([P, NQ, 1], F32, name="negmax", tag="negmax")
                S_sum = stats.tile([P, NQ, 1], F32, name="S_sum", tag="S")
                invS = stats.tile([P, NQ, 1], F32, name="invS", tag="invS")
                attnT = [
                    atpool.tile([P, NKC, P], F16, name=f"attnT{i}_{r}", tag=f"attnT{r}")
                    for r in range(NQ)
                ]
                ctx16 = ctxpool.tile([P, NQ, D], F16, name=f"ctx{i}", tag="ctx")
                ps_scores = [None] * NQ

                def scores_mm(r, kh):
                    if ps_scores[r] is None:
                        ps_scores[r] = psbig.tile([P, TB], F32, name="ps_s", tag="psb")
                    ps_s = ps_scores[r]
                    for j in range(NDC):
                        nc.tensor.matmul(
                            ps_s[:, kh * 512 : (kh + 1) * 512],
                            hWT[i][:, j, r * P : (r + 1) * P],
                            bT[i][:, j, kh * 512 : (kh + 1) * 512],
                            start=(j == 0),
                            stop=(j == NDC - 1),
                        )

                def softmax_half(r):
                    ps_s = ps_scores[r]
                    nc.vector.tensor_reduce(
                        negmax[:, r, :],
                        ps_s[:],
                        axis=mybir.AxisListType.X,
                        op=mybir.AluOpType.max,
                        negate=True,
                    )
                    nc.scalar.activation(
                        E[:, r, :],
                        ps_s[:],
                        mybir.ActivationFunctionType.Exp,
                        bias=negmax[:, r, :],
                        accum_out=S_sum[:, r, :],
                    )
                    nc.vector.reciprocal(invS[:, r, :], S_sum[:, r, :])

                def attnT_half(r):
                    ps = psT.tile([P, TB], F16, name="ps_at", tag="ps16")
                    for c in range(NKC):
                        nc.tensor.transpose(
                            ps[:, c * P : (c + 1) * P],
                            E[:, r, c * P : (c + 1) * P],
                            ident16,
                        )
                    nc.vector.tensor_copy(
                        attnT[r][:].rearrange("p a b -> p (a b)"),
                        ps[:],
                    )

                def ctx_mm(r, split=False):
                    ps_c = psbig.tile([P, D], F32, name="ps_c", tag="psb")
                    for dh in range(2):
                        for c in range(NKC):
                            nc.tensor.matmul(
                                ps_c[:, dh * 512 : (dh + 1) * 512],
                                attnT[r][:, c, :],
                                bN[i][:, c, dh * 512 : (dh + 1) * 512],
                                start=(c == 0),
                                stop=(c == NKC - 1),
                            )
                        if split:
                            # epilogue per 512-col half: the scale + out DMA
                            # of half 0 overlap the PE matmuls of half 1
                            sl = slice(dh * 512, (dh + 1) * 512)
                            nc.scalar.mul(ctx16[:, r, sl], ps_c[:, sl], invS[:, r, :])
                            nc.sync.dma_start(
                                out_ext[i, r * P : (r + 1) * P, sl], ctx16[:, r, sl]
                            )
                    if not split:
                        nc.scalar.mul(ctx16[:, r, :], ps_c[:], invS[:, r, :])
                        nc.scalar.dma_start(
                            out_ext[i, r * P : (r + 1) * P, :], ctx16[:, r, :]
                        )

                return scores_mm, softmax_half, attnT_half, ctx_mm

            def emit_loads(i):
                if i + 1 < BPC:
                    bT[i + 1] = load_bt(i + 1, halves=False)
                    bN[i + 1] = load_b(i + 1)
                if i + 2 < BPC:
                    hT[i + 2] = load_ht(i + 2)
                if i + 1 < BPC:
                    hWT[i + 1] = hwtpool.tile(
                        [P, NDC, TH], F16, name=f"hWT{i+1}", tag="hWT"
                    )

            # Batches 0..1: ctx interleaved with next batch's hWT groups.
            ops = {}
            for i in range(2):
                scores_mm, softmax_half, attnT_half, ctx_mm = make_batch(i)
                emit_loads(i)
                scores_mm(0, 0)
                scores_mm(0, 1)
                softmax_half(0)
                scores_mm(1, 0)
                scores_mm(1, 1)
                attnT_half(0)
                softmax_half(1)
                emit_hwt_group(i + 1, 0)
                ctx_mm(0)
                attnT_half(1)
                emit_hwt_group(i + 1, 2)
                ctx_mm(1)
                emit_hwt_group(i + 1, 4)
                emit_hwt_group(i + 1, 6)

            # Batch 2: compute only; its ctx is deferred into batch 3's
            # stream so the final softmax/copy latency hides behind PE work.
            s2, sm2, at2, ctx2 = make_batch(2)
            emit_loads(2)
            s2(0, 0)
            s2(0, 1)
            sm2(0)
            s2(1, 0)
            s2(1, 1)
            at2(0)
            sm2(1)
            emit_hwt_group(3, 0)
            emit_hwt_group(3, 2)
            at2(1)
            emit_hwt_group(3, 4)
            emit_hwt_group(3, 6)

            # Batch 3: scores/softmax/attnT, with batch 2's ctx as filler,
            # then its own ctx with split epilogues to shorten the drain.
            s3, sm3, at3, ctx3 = make_batch(3)
            s3(0, 0)
            s3(0, 1)
            sm3(0)
            s3(1, 0)
            s3(1, 1)
            at3(0)
            sm3(1)
            ctx2(0)
            at3(1)
            ctx2(1)
            ctx3(0, split=True)
            ctx3(1, split=True)
    _split_excess_waits(nc)
    return nc


_NC_CACHE = None


def _get_nc():
    global _NC_CACHE
    if _NC_CACHE is None:
        _NC_CACHE = build_nc()
    return _NC_CACHE


def run(b, h, W_b, trace=False):
    """Shard, execute on 8 cores, gather. Returns (ctx, BassKernelResults)."""
    assert b.shape == (B, TB, D) and h.shape == (B, TH, D)
    # All on-chip compute is fp16; cast and pre-transpose on the host so the
    # PE never spends cycles on layout changes.
    W16 = np.ascontiguousarray(W_b[0].astype(np.float16))
    b16 = b.astype(np.float16)
    bT16 = np.ascontiguousarray(b16.transpose(0, 2, 1))
    hT16 = np.ascontiguousarray(h.astype(np.float16).transpose(0, 2, 1))
    b16 = np.ascontiguousarray(b16)
    ident = np.eye(P, dtype=np.float16)
    in_maps = []
    for c in range(N_CORES):
        sl = slice(c * BPC, (c + 1) * BPC)
        in_maps.append(
            {
                "b": b16[sl],
                "bT": bT16[sl],
                "hT": hT16[sl],
                "w": W16,
                "ident": ident,
            }
        )
    res = run_bass_kernel_spmd(
        _get_nc(), in_maps, core_ids=list(range(N_CORES)), trace=trace
    )
    out = np.concatenate([res.results[c]["out"] for c in range(N_CORES)], axis=0)
    return out.astype(np.float32), res


def kernel(b, h, W_b):
    out, _ = run(b, h, W_b, trace=False)
    return out


# revision 8
# speedup vs baseline: 1.1661x; 1.1661x over previous
"""Trainium2 Bass kernel for nn_Attention_3513283248742.

Bilinear attention: scores = h @ W @ b^T, attn = softmax(scores, -1),
ctx = attn @ b.  Shapes: b [32,1024,1024], h [32,256,1024], W_b [1,1024,1024].

Sharding: data-parallel over batch B=32 across 8 NeuronCores (4 batches per
core); W replicated.  No collectives.

v2 over the previous kernel: all hT / bT transposes move to the HOST (the
inputs are sent both in natural and transposed layout, fp16), removing 80
PE transposes per batch (~17us/core of PE time).  Output DMA'd as fp16 and
upcast on the host.  Per-core DMA grows to ~22MB (~62us) which stays under
the fp16 matmul floor (~85us), so the kernel remains PE-bound.

Per-core pipeline (per batch i):
  hWT  = W^T @ hT_i           lhsT = W chunks, rhs = hT (from host)
  S    = hWT^T @ bT_i         scores [q,k]
  softmax over k: DVE row max, ACT exp (+rowsum via accum), DVE recip
  attnT = PE transposes of E (the only transposes left on the PE)
  ctx  = attnT^T @ b_i        rhs = b natural layout (from host)
  out  = ctx * invS           ACT epilogue, fp16, DMA'd from the ACT queue

hWT(i+1) matmul groups are interleaved into batch i's attnT/ctx stream as
PE filler so softmax / PSUM->SBUF copy latency never stalls the PE.  Batch
0's hWT runs j-outer (contraction-chunk outer) so its matmuls chase the W
chunk DMAs during the ramp.
"""

import numpy as np

import concourse.bass as bass
import concourse.mybir as mybir
import concourse.tile as tile
from concourse.bass_utils import run_bass_kernel_spmd
from concourse.vector_clock import ScopedClock

F32 = mybir.dt.float32
F16 = mybir.dt.float16

N_CORES = 8
B, TB, TH, D = 32, 1024, 256, 1024
BPC = B // N_CORES  # batches per core = 4
P = 128
NDC = D // P   # 8 chunks of the D axis
NKC = TB // P  # 8 chunks of the k axis
NQ = TH // P   # 2 chunks of the q axis

_PATCHED = False
CLEAR_SEMS_ON_EXIT = False


def _patch_tile_drain(max_waits_per_inst: int = 1):
    """This walrus build rejects >1 sem wait on the SP Drain instruction that
    TileContext emits on exit; split the waits across preceding sync nops."""
    global _PATCHED
    if _PATCHED:
        return
    _PATCHED = True

    def _drain_and_barrier(self, tick_clock, wait_clock):
        nc = self.nc
        drain_inst = nc.sync.drain()
        wait_clock.add_sem_waits(
            drain_inst.ins, ScopedClock({None: tick_clock.global_clock})
        )
        si = drain_inst.ins.sync_info
        if si is not None and si.on_wait and len(si.on_wait) > max_waits_per_inst:
            waits = list(si.on_wait)
            bb = nc.cur_bb.bb
            assert bb.instructions[-1] is drain_inst.ins
            bb.instructions.pop()
            si.on_wait = waits[:max_waits_per_inst]
            rest = waits[max_waits_per_inst:]
            for i in range(0, len(rest), max_waits_per_inst):
                nop = nc.sync.nop(nofuse=True)
                chunk = rest[i : i + max_waits_per_inst]
                if nop.ins.sync_info is None:
                    nop.ins.sync_info = mybir.SyncInfo(on_wait=chunk, on_update=[])
                else:
                    nop.ins.sync_info.on_wait.extend(chunk)
            bb.instructions.append(drain_inst.ins)
        nc.all_engine_barrier()
        assert self.sems is not None
        popped = nc._tile_sem_poison_stack.pop()
        assert popped is self._sem_poison
        if CLEAR_SEMS_ON_EXIT:
            nc.clear_and_free_semaphores(list(self.sems.allocated().values()))
            nc.all_engine_barrier()
        else:
            nc._state.prepend_free_semaphores(
                [
                    s.num if hasattr(s, "num") else s
                    for s in self.sems.allocated().values()
                ]
            )

    tile.TileContext._drain_and_barrier = _drain_and_barrier


def _split_excess_waits(nc, max_waits: int = 1):
    """Walrus rejects instructions carrying more than `max_waits` sem waits.
    Hoist excess waits onto same-engine nops inserted just before."""
    for f in nc.m.functions:
        for bb in f.blocks:
            out = []
            for ins in list(bb.instructions):
                si = ins.sync_info
                if si is not None and si.on_wait and len(si.on_wait) > max_waits:
                    waits = list(si.on_wait)
                    si.on_wait = waits[:max_waits]
                    rest = waits[max_waits:]
                    for i in range(0, len(rest), max_waits):
                        nop = nc.engines[ins.engine].nop(nofuse=True)
                        cur_bb = nc.cur_bb.bb
                        assert cur_bb.instructions[-1] is nop.ins
                        cur_bb.instructions.pop()
                        nop.ins.sync_info = mybir.SyncInfo(
                            on_wait=rest[i : i + max_waits], on_update=[]
                        )
                        out.append(nop.ins)
                out.append(ins)
            bb.instructions[:] = out


def build_nc():
    _patch_tile_drain()
    nc = bass.Bass(trn_type="TRN2", target_bir_lowering=False, debug=False)
    b_ext = nc.declare_dram_parameter("b", [BPC, TB, D], F16, isOutput=False)
    bt_ext = nc.declare_dram_parameter("bT", [BPC, D, TB], F16, isOutput=False)
    ht_ext = nc.declare_dram_parameter("hT", [BPC, D, TH], F16, isOutput=False)
    w_ext = nc.declare_dram_parameter("w", [D, D], F16, isOutput=False)
    ident_ext = nc.declare_dram_parameter("ident", [P, P], F16, isOutput=False)
    out_ext = nc.declare_dram_parameter("out", [BPC, TH, D], F16, isOutput=True)

    with tile.TileContext(nc) as tc:
        with (
            tc.tile_pool(name="consts", bufs=1) as consts,
            tc.tile_pool(name="bpool", bufs=2) as bpool,
            tc.tile_pool(name="btpool", bufs=2) as btpool,
            tc.tile_pool(name="htpool", bufs=2) as htpool,
            tc.tile_pool(name="hwtpool", bufs=2) as hwtpool,
            tc.tile_pool(name="epool", bufs=2) as epool,
            tc.tile_pool(name="atpool", bufs=2) as atpool,
            tc.tile_pool(name="ctxpool", bufs=2) as ctxpool,
            tc.tile_pool(name="stats", bufs=2) as stats,
            tc.tile_pool(name="psbig", bufs=2, space="PSUM") as psbig,
            tc.tile_pool(name="pshw", bufs=2, space="PSUM") as pshw,
            tc.tile_pool(name="psT", bufs=2, space="PSUM") as psT,
        ):
            # ident on the (otherwise idle at t=0) scalar DMA queue so the
            # warmup transposes can start as soon as the preamble ends.
            ident_t = consts.tile([P, P], F16)
            nc.scalar.dma_start(ident_t[:], ident_ext.ap())
            ident16 = ident_t[:]

            w16 = consts.tile([P, NDC, D], F16)  # [din(part), j, dout]

            # --- DMA emission helpers (sync queue = priority load stream) ---
            def load_ht(i):
                t = htpool.tile([P, NDC, TH], F16, name=f"hT{i}", tag="hT")
                nc.sync.dma_start(t[:], ht_ext[i].rearrange("(c p) q -> p c q", p=P))
                return t

            def load_bt(i, halves):
                t = btpool.tile([P, NDC, TB], F16, name=f"bT{i}", tag="bT")
                if halves:
                    for kh in range(2):
                        nc.sync.dma_start(
                            t[:, :, kh * 512 : (kh + 1) * 512],
                            bt_ext[i, :, kh * 512 : (kh + 1) * 512].rearrange(
                                "(c p) k -> p c k", p=P
                            ),
                        )
                else:
                    nc.sync.dma_start(
                        t[:], bt_ext[i].rearrange("(c p) k -> p c k", p=P)
                    )
                return t

            def load_b(i):
                t = bpool.tile([P, NKC, D], F16, name=f"b{i}", tag="b")
                nc.sync.dma_start(t[:], b_ext[i].rearrange("(c p) d -> p c d", p=P))
                return t

            # --- ramp: priority-ordered loads on the sync queue ---
            hT = [None] * (BPC + 1)
            bT = [None] * BPC
            bN = [None] * BPC
            hT[0] = load_ht(0)
            for j2 in range(4):  # W in 4 chunks so hWT(0) can chase arrivals
                nc.sync.dma_start(
                    w16[:, 2 * j2 : 2 * j2 + 2, :],
                    w_ext[j2 * 256 : (j2 + 1) * 256, :].rearrange(
                        "(c p) d -> p c d", p=P
                    ),
                )
            bT[0] = load_bt(0, halves=True)
            hT[1] = load_ht(1)
            bN[0] = load_b(0)

            # --- PE warmup: ramp the clock while the ramp DMAs stream ---
            for wi in range(12):
                wt = psT.tile([P, TB], F16, name="warm", tag="ps16")
                nc.tensor.transpose(
                    wt[:, (wi % 8) * P : (wi % 8 + 1) * P], ident16, ident16
                )

            hWT = [None] * (BPC + 1)
            hWT[0] = hwtpool.tile([P, NDC, TH], F16, name="hWT0", tag="hWT")

            def emit_hwt_group(i, tp):
                """One tp-group (2 dout chunks) of hWT for batch i. 16 mm."""
                ps = pshw.tile([P, 512], F32, name="ps_hw", tag="pshw")
                for dt in range(2):
                    t = tp + dt
                    for j in range(NDC):
                        nc.tensor.matmul(
                            ps[:, dt * 256 : (dt + 1) * 256],
                            w16[:, j, t * P : (t + 1) * P],
                            hT[i][:, j, :],
                            start=(j == 0),
                            stop=(j == NDC - 1),
                        )
                nc.vector.tensor_copy(
                    hWT[i][:, tp : tp + 2, :].rearrange("p a b -> p (a b)"),
                    ps[:],
                )

            # hWT for batch 0 during the ramp: contiguous accumulation
            # groups; each matmul's W-chunk DMA dependency still lets the
            # stream chase the W arrivals.
            for tp in range(0, NDC, 2):
                emit_hwt_group(0, tp)

            # --- per-batch emission ---
            def make_batch(i):
                E = epool.tile([P, NQ, TB], F16, name=f"E{i}", tag="E")
                negmax = stats.tile([P, NQ, 1], F32, name="negmax", tag="negmax")
                S_sum = stats.tile([P, NQ, 1], F32, name="S_sum", tag="S")
                invS = stats.tile([P, NQ, 1], F32, name="invS", tag="invS")
                attnT = [
                    atpool.tile([P, NKC, P], F16, name=f"attnT{i}_{r}", tag=f"attnT{r}")
                    for r in range(NQ)
                ]
                ctx16 = ctxpool.tile([P, NQ, D], F16, name=f"ctx{i}", tag="ctx")
                ps_scores = [None] * NQ

                def scores_mm(r, kh):
                    if ps_scores[r] is None:
                        ps_scores[r] = psbig.tile([P, TB], F32, name="ps_s", tag="psb")
                    ps_s = ps_scores[r]
                    for j in range(NDC):
                        nc.tensor.matmul(
                            ps_s[:, kh * 512 : (kh + 1) * 512],
                            hWT[i][:, j, r * P : (r + 1) * P],
                            bT[i][:, j, kh * 512 : (kh + 1) * 512],
                            start=(j == 0),
                            stop=(j == NDC - 1),
                        )

                def softmax_half(r):
                    ps_s = ps_scores[r]
                    nc.vector.tensor_reduce(
                        negmax[:, r, :],
                        ps_s[:],
                        axis=mybir.AxisListType.X,
                        op=mybir.AluOpType.max,
                        negate=True,
                    )
                    nc.scalar.activation(
                        E[:, r, :],
                        ps_s[:],
                        mybir.ActivationFunctionType.Exp,
                        bias=negmax[:, r, :],
                        accum_out=S_sum[:, r, :],
                    )
                    nc.vector.reciprocal(invS[:, r, :], S_sum[:, r, :])

                def attnT_half(r):
                    ps = psT.tile([P, TB], F16, name="ps_at", tag="ps16")
                    for c in range(NKC):
                        nc.tensor.transpose(
                            ps[:, c * P : (c + 1) * P],
                            E[:, r, c * P : (c + 1) * P],
                            ident16,
                        )
                    nc.vector.tensor_copy(
                        attnT[r][:].rearrange("p a b -> p (a b)"),
                        ps[:],
                    )

                def ctx_mm(r, split=False):
                    if split:
                        # separate [P,512] PSUM tiles per half: the half-0
                        # epilogue (mul reads PSUM) must not carry a
                        # tile-granular WAR against the half-1 matmuls
                        for dh in range(2):
                            ps_h = pshw.tile([P, 512], F32, name="ps_cs", tag="pshw")
                            for c in range(NKC):
                                nc.tensor.matmul(
                                    ps_h[:],
                                    attnT[r][:, c, :],
                                    bN[i][:, c, dh * 512 : (dh + 1) * 512],
                                    start=(c == 0),
                                    stop=(c == NKC - 1),
                                )
                            sl = slice(dh * 512, (dh + 1) * 512)
                            nc.scalar.mul(ctx16[:, r, sl], ps_h[:], invS[:, r, :])
                            nc.sync.dma_start(
                                out_ext[i, r * P : (r + 1) * P, sl], ctx16[:, r, sl]
                            )
                        return
                    ps_c = psbig.tile([P, D], F32, name="ps_c", tag="psb")
                    for dh in range(2):
                        for c in range(NKC):
                            nc.tensor.matmul(
                                ps_c[:, dh * 512 : (dh + 1) * 512],
                                attnT[r][:, c, :],
                                bN[i][:, c, dh * 512 : (dh + 1) * 512],
                                start=(c == 0),
                                stop=(c == NKC - 1),
                            )
                    nc.scalar.mul(ctx16[:, r, :], ps_c[:], invS[:, r, :])
                    nc.scalar.dma_start(
                        out_ext[i, r * P : (r + 1) * P, :], ctx16[:, r, :]
                    )

                return scores_mm, softmax_half, attnT_half, ctx_mm

            def emit_loads(i):
                if i + 1 < BPC:
                    bT[i + 1] = load_bt(i + 1, halves=False)
                    bN[i + 1] = load_b(i + 1)
                if i + 2 < BPC:
                    hT[i + 2] = load_ht(i + 2)
                if i + 1 < BPC:
                    hWT[i + 1] = hwtpool.tile(
                        [P, NDC, TH], F16, name=f"hWT{i+1}", tag="hWT"
                    )

            # Batches 0..1: ctx interleaved with next batch's hWT groups.
            ops = {}
            for i in range(2):
                scores_mm, softmax_half, attnT_half, ctx_mm = make_batch(i)
                emit_loads(i)
                scores_mm(0, 0)
                scores_mm(0, 1)
                softmax_half(0)
                scores_mm(1, 0)
                scores_mm(1, 1)
                attnT_half(0)
                softmax_half(1)
                emit_hwt_group(i + 1, 0)
                ctx_mm(0)
                attnT_half(1)
                emit_hwt_group(i + 1, 2)
                ctx_mm(1)
                emit_hwt_group(i + 1, 4)
                emit_hwt_group(i + 1, 6)

            # Batch 2: compute only; its ctx is deferred into batch 3's
            # stream so the final softmax/copy latency hides behind PE work.
            s2, sm2, at2, ctx2 = make_batch(2)
            emit_loads(2)
            s2(0, 0)
            s2(0, 1)
            sm2(0)
            s2(1, 0)
            s2(1, 1)
            at2(0)
            sm2(1)
            emit_hwt_group(3, 0)
            emit_hwt_group(3, 2)
            at2(1)
            emit_hwt_group(3, 4)
            emit_hwt_group(3, 6)

            # Batch 3: scores/softmax/attnT, with batch 2's ctx as filler,
            # then its own ctx with split epilogues to shorten the drain.
            s3, sm3, at3, ctx3 = make_batch(3)
            s3(0, 0)
            s3(0, 1)
            sm3(0)
            s3(1, 0)
            s3(1, 1)
            at3(0)
            sm3(1)
            ctx2(0)
            at3(1)
            ctx2(1)
            ctx3(0, split=True)
            ctx3(1, split=True)
    _split_excess_waits(nc)
    return nc


_NC_CACHE = None


def _get_nc():
    global _NC_CACHE
    if _NC_CACHE is None:
        _NC_CACHE = build_nc()
    return _NC_CACHE


def run(b, h, W_b, trace=False):
    """Shard, execute on 8 cores, gather. Returns (ctx, BassKernelResults)."""
    assert b.shape == (B, TB, D) and h.shape == (B, TH, D)
    # All on-chip compute is fp16; cast and pre-transpose on the host so the
    # PE never spends cycles on layout changes.
    W16 = np.ascontiguousarray(W_b[0].astype(np.float16))
    b16 = b.astype(np.float16)
    bT16 = np.ascontiguousarray(b16.transpose(0, 2, 1))
    hT16 = np.ascontiguousarray(h.astype(np.float16).transpose(0, 2, 1))
    b16 = np.ascontiguousarray(b16)
    ident = np.eye(P, dtype=np.float16)
    in_maps = []
    for c in range(N_CORES):
        sl = slice(c * BPC, (c + 1) * BPC)
        in_maps.append(
            {
                "b": b16[sl],
                "bT": bT16[sl],
                "hT": hT16[sl],
                "w": W16,
                "ident": ident,
            }
        )
    res = run_bass_kernel_spmd(
        _get_nc(), in_maps, core_ids=list(range(N_CORES)), trace=trace
    )
    out = np.concatenate([res.results[c]["out"] for c in range(N_CORES)], axis=0)
    return out.astype(np.float32), res


def kernel(b, h, W_b):
    out, _ = run(b, h, W_b, trace=False)
    return out
